# revision 1
# baseline (speedup 1.0000x reference)
"""MultiHeadAttention Trainium2 kernel (8 NeuronCores, SPMD).

Sharding: core c = (batch b=c//2, head-group g=c%2). Each core computes
8 of the 16 heads for one batch: Q/K/V projections restricted to the
512 d_model dims of its head group, full attention for those heads, and
a partial output projection. Host sums the two head-group partials per
batch and adds the output bias.

Device-side layout (all bf16, f32 accumulation in PSUM):
  xq/xk/xv : (1024, 2048)  = x[b].T          (d_in on partitions)
  wq/wk/wv : (1024, 512)   = W[rows g].T     (d_in, d_out_local)
  wo       : (512, 1024)   = Wo[:, cols g].T (d_in_local, d_out)
  qhT/khT  : (512, 2048)   head-transposed activations (d_out, tokens)
  vh       : (2048, 520)   v-heads interleaved with a ones column per
                           head -> ctx matmul also produces softmax
                           denominators (row 64 of each ctx PSUM tile).
Scores are computed transposed (k on partitions, q free) so the exp'd
tile P^T feeds the ctx matmul directly with no transposes of P.
"""

import numpy as np
import ml_dtypes

BF16 = ml_dtypes.bfloat16

B, S, D, H = 4, 2048, 1024, 16
DH = 64          # head dim
DL = 512         # local d_out (8 heads)
P = 128          # partitions
NCORES = 8
SCALE = 1.0 / np.sqrt(DH)

_CACHE = {}
LAST_RESULTS = None  # stashed BassKernelResults for test harness


def _build_nc():
    import concourse.bass as bass
    from concourse import bacc, mybir
    from concourse.tile import TileContext

    f32 = mybir.dt.float32
    bf16 = mybir.dt.bfloat16

    nc = bacc.Bacc("TRN2", target_bir_lowering=False, debug=False, num_devices=NCORES)

    # pre-tiled on host so every activation DMA is one contiguous block
    xq_d = nc.dram_tensor("xq", (8, 4, P, 512), bf16, kind="ExternalInput")
    xk_d = nc.dram_tensor("xk", (8, 4, P, 512), bf16, kind="ExternalInput")
    xv_d = nc.dram_tensor("xv", (8, 16, P, P), bf16, kind="ExternalInput")
    wq_d = nc.dram_tensor("wq", (D, DL), bf16, kind="ExternalInput")
    wk_d = nc.dram_tensor("wk", (D, DL), bf16, kind="ExternalInput")
    wv_d = nc.dram_tensor("wv", (D, DL), bf16, kind="ExternalInput")
    wo_d = nc.dram_tensor("wo", (DL, D), bf16, kind="ExternalInput")
    bq_d = nc.dram_tensor("bq", (P, 4), f32, kind="ExternalInput")
    bk_d = nc.dram_tensor("bk", (P, 4), f32, kind="ExternalInput")
    bvb_d = nc.dram_tensor("bvb", (P, 520), bf16, kind="ExternalInput")
    out_d = nc.dram_tensor("out", (S, D), bf16, kind="ExternalOutput")

    Exp = mybir.ActivationFunctionType.Exp

    with TileContext(nc) as tc:
        # ---------------- resident SBUF ----------------
        with tc.tile_pool(name="res", bufs=1) as res:
            wq_sb = res.tile([P, 8, DL], bf16)
            wk_sb = res.tile([P, 8, DL], bf16)
            wv_sb = res.tile([P, 8, DL], bf16)
            wo_sb = res.tile([P, 4, D], bf16)
            bq_sb = res.tile([P, 4], f32)
            bk_sb = res.tile([P, 4], f32)
            bvb_sb = res.tile([P, 520], bf16)
            nc.sync.dma_start(out=wq_sb, in_=wq_d.ap().rearrange("(c p) d -> p c d", p=P))
            nc.sync.dma_start(out=wk_sb, in_=wk_d.ap().rearrange("(c p) d -> p c d", p=P))
            nc.sync.dma_start(out=wv_sb, in_=wv_d.ap().rearrange("(c p) d -> p c d", p=P))
            nc.sync.dma_start(out=wo_sb, in_=wo_d.ap().rearrange("(c p) d -> p c d", p=P))
            nc.sync.dma_start(out=bq_sb, in_=bq_d.ap())
            nc.sync.dma_start(out=bk_sb, in_=bk_d.ap())
            nc.sync.dma_start(out=bvb_sb, in_=bvb_d.ap())

            # activations (resident through whole kernel)
            qhT = [res.tile([P, S], bf16, name=f"qhT{i}") for i in range(4)]
            khT = [res.tile([P, S], bf16, name=f"khT{i}") for i in range(4)]
            vh = [res.tile([P, 8 * 65], bf16, name=f"vh{i}") for i in range(16)]
            ctxT = [res.tile([P, S], bf16, name=f"ctxT{i}") for i in range(4)]

            # Projections for pair p+1 are emitted interleaved with the
            # attention of pair p: the chained projection matmuls fill the
            # TensorE between exp-gated attention matmuls, keeping PE busy
            # enough that the HAM clock gate stays at full rate.
            with tc.tile_pool(name="xs", bufs=6) as xs, \
                 tc.tile_pool(name="pj", bufs=2, space="PSUM") as pj, \
                 tc.tile_pool(name="sc", bufs=2, space="PSUM") as sc, \
                 tc.tile_pool(name="cx", bufs=1, space="PSUM") as cx, \
                 tc.tile_pool(name="pt", bufs=4) as ptp, \
                 tc.tile_pool(name="sm", bufs=2) as sm:

                def proj_qk_unit(x_d, w_sb, b_sb, dst, p, t):
                    # one token-tile of the Q or K projection for pair p
                    ps = pj.tile([P, 512], f32, name="pjt", tag="pj")
                    for c in range(8):
                        xt = xs.tile([P, 512], bf16, name="xt", tag="x")
                        nc.sync.dma_start(out=xt, in_=x_d.ap()[c, t])
                        nc.tensor.matmul(
                            ps, lhsT=w_sb[:, c, p * P:(p + 1) * P], rhs=xt,
                            start=(c == 0), stop=(c == 7))
                    nc.vector.tensor_scalar_add(
                        out=dst[p][:, t * 512:(t + 1) * 512],
                        in0=ps, scalar1=b_sb[:, p:p + 1])

                def proj_v_unit(p, tb):
                    # one token-block of the V projection (2 heads of pair p)
                    psv = pj.tile([P, P], f32, name="psv", tag="pj")
                    for c in range(8):
                        xvt = xs.tile([P, P], bf16, name="xvt", tag="xv")
                        nc.sync.dma_start(out=xvt, in_=xv_d.ap()[c, tb])
                        nc.tensor.matmul(
                            psv, lhsT=xvt, rhs=wv_sb[:, c, p * P:(p + 1) * P],
                            start=(c == 0), stop=(c == 7))
                    vt = vh[tb].rearrange("p (h e) -> p h e", e=65)
                    nc.vector.tensor_copy(
                        vt[:, 2 * p:2 * p + 2, 0:64],
                        psv.rearrange("p (h e) -> p h e", e=64))
                    nc.gpsimd.memset(vt[:, 2 * p:2 * p + 2, 64:65], 1.0)
                    nc.vector.tensor_add(
                        vh[tb][:, 130 * p:130 * (p + 1)],
                        vh[tb][:, 130 * p:130 * (p + 1)],
                        bvb_sb[:, 130 * p:130 * (p + 1)])

                def proj_units(p):
                    units = []
                    for t in range(4):
                        units.append(lambda t=t: proj_qk_unit(xq_d, wq_sb, bq_sb, qhT, p, t))
                    for t in range(4):
                        units.append(lambda t=t: proj_qk_unit(xk_d, wk_sb, bk_sb, khT, p, t))
                    for tb in range(16):
                        units.append(lambda tb=tb: proj_v_unit(p, tb))
                    return units

                def attn_pair(p, filler):
                    # filler: iterator of proj units for pair p+1, drained
                    # gradually so their program-order priority interleaves
                    # them with the exp-gated attention matmuls.
                    fi = 0
                    for hh in range(2):
                        h = 2 * p + hh
                        po = 64 * hh
                        for qh in range(2):       # q halves of 1024
                            q0 = qh * 1024
                            cps = cx.tile([P, 1024], f32, name="cps", tag="cx")
                            for kb in range(16):  # k blocks of 128
                                if kb % 3 == 0 and fi < len(filler):
                                    filler[fi]()
                                    fi += 1
                                sps = sc.tile([P, 1024], f32, name="sps", tag="s")
                                for j in range(2):
                                    nc.tensor.matmul(
                                        sps[:, j * 512:(j + 1) * 512],
                                        lhsT=khT[p][po:po + 64, kb * P:(kb + 1) * P],
                                        rhs=qhT[p][po:po + 64, q0 + j * 512:q0 + (j + 1) * 512],
                                        start=True, stop=True)
                                pt = ptp.tile([P, 1024], bf16, name="ptt", tag="pt")
                                nc.scalar.activation(pt, sps, Exp, scale=SCALE)
                                for j in range(2):
                                    nc.tensor.matmul(
                                        cps[0:65, j * 512:(j + 1) * 512],
                                        lhsT=vh[kb][:, 65 * h:65 * h + 65],
                                        rhs=pt[:, j * 512:(j + 1) * 512],
                                        start=(kb == 0), stop=(kb == 15))
                            # normalize: row 64 holds sum_k P
                            rc = sm.tile([1, 1024], f32, name="rc", tag="rc")
                            nc.vector.reciprocal(rc, cps[64:65, :])
                            bc = sm.tile([64, 1024], f32, name="bc", tag="bc")
                            step = list(rc.ap[1])[0] if hasattr(rc.ap[1], "__iter__") else 1
                            nc.gpsimd.dma_start(
                                out=bc,
                                in_=bass.AP(tensor=rc.tensor, offset=rc.offset,
                                            ap=[[1, 1], [0, 64], [step, 1024]]))
                            nc.vector.tensor_mul(
                                ctxT[p][po:po + 64, q0:q0 + 1024], cps[0:64, :], bc)
                    # drain any remaining filler units
                    while fi < len(filler):
                        filler[fi]()
                        fi += 1

                for u in proj_units(0):
                    u()
                for p in range(4):
                    attn_pair(p, proj_units(p + 1) if p + 1 < 4 else [])

            # ---------------- phase 3: output projection ----------------
            with tc.tile_pool(name="po", bufs=3, space="PSUM") as pop, \
                 tc.tile_pool(name="ot", bufs=3) as otp:
                for qb in range(16):
                    ops = pop.tile([P, D], f32, name="ops", tag="po")
                    for p in range(4):
                        for n in range(2):
                            nc.tensor.matmul(
                                ops[:, n * 512:(n + 1) * 512],
                                lhsT=ctxT[p][:, qb * P:(qb + 1) * P],
                                rhs=wo_sb[:, p, n * 512:(n + 1) * 512],
                                start=(p == 0), stop=(p == 3))
                    ot = otp.tile([P, D], bf16, name="ot", tag="ot")
                    nc.vector.tensor_copy(ot, ops)
                    nc.gpsimd.dma_start(out=out_d.ap()[qb * P:(qb + 1) * P, :], in_=ot)

    nc.finalize()
    return nc


def _prep_in_maps(q, k, v, Wq, bq, Wk, bk, Wv, bv, Wo, bo):
    in_maps = []
    for c in range(NCORES):
        b, g = c // 2, c % 2
        sl = slice(g * DL, (g + 1) * DL)
        bvl = np.asarray(bv)[sl].astype(np.float32)
        bvb = np.zeros(520, np.float32)
        for h in range(8):
            bvb[65 * h:65 * h + 64] = bvl[64 * h:64 * h + 64]
        bvb = np.broadcast_to(bvb, (P, 520))
        def tile_qk(x):
            xt = np.asarray(x)[b].T.astype(BF16)          # (1024, 2048)
            return np.ascontiguousarray(
                xt.reshape(8, P, 4, 512).transpose(0, 2, 1, 3))
        def tile_v(x):
            xt = np.asarray(x)[b].T.astype(BF16)
            return np.ascontiguousarray(
                xt.reshape(8, P, 16, P).transpose(0, 2, 1, 3))
        in_maps.append({
            "xq": tile_qk(q),
            "xk": tile_qk(k),
            "xv": tile_v(v),
            "wq": np.ascontiguousarray(np.asarray(Wq)[sl, :].T).astype(BF16),
            "wk": np.ascontiguousarray(np.asarray(Wk)[sl, :].T).astype(BF16),
            "wv": np.ascontiguousarray(np.asarray(Wv)[sl, :].T).astype(BF16),
            "wo": np.ascontiguousarray(np.asarray(Wo)[:, sl].T).astype(BF16),
            "bq": np.ascontiguousarray(np.asarray(bq)[sl].reshape(4, P).T).astype(np.float32),
            "bk": np.ascontiguousarray(np.asarray(bk)[sl].reshape(4, P).T).astype(np.float32),
            "bvb": np.ascontiguousarray(bvb).astype(BF16),
        })
    return in_maps


def _get_runner():
    """Build nc + jitted SPMD executor once; reuse across kernel() calls."""
    if "runner" in _CACHE:
        return _CACHE["runner"]
    import jax
    import jax.numpy as jnp
    from jax.sharding import Mesh, PartitionSpec
    from jax.experimental.shard_map import shard_map
    from concourse import mybir
    from concourse.bass2jax import (_bass_exec_p, install_neuronx_cc_hook,
                                    partition_id_tensor)

    nc = _build_nc()
    install_neuronx_cc_hook()

    partition_name = nc.partition_id_tensor.name if nc.partition_id_tensor else None
    in_names, out_names, out_avals, zero_shapes = [], [], [], []
    for alloc in nc.m.functions[0].allocations:
        if not isinstance(alloc, mybir.MemoryLocationSet):
            continue
        name = alloc.memorylocations[0].name
        if alloc.kind == "ExternalInput":
            if name != partition_name:
                in_names.append(name)
        elif alloc.kind == "ExternalOutput":
            shape = tuple(alloc.tensor_shape)
            dtype = mybir.dt.np(alloc.dtype)
            out_names.append(name)
            out_avals.append(jax.core.ShapedArray(shape, dtype))
            zero_shapes.append((shape, dtype))
    n_params = len(in_names)
    all_in_names = list(in_names) + list(out_names)
    if partition_name is not None:
        all_in_names.append(partition_name)

    def _body(*args):
        operands = list(args)
        if partition_name is not None:
            operands.append(partition_id_tensor())
        outs = _bass_exec_p.bind(
            *operands,
            out_avals=tuple(out_avals),
            in_names=tuple(all_in_names),
            out_names=tuple(out_names),
            lowering_input_output_aliases=(),
            sim_require_finite=True,
            sim_require_nnan=True,
            nc=nc,
        )
        return tuple(outs)

    devices = jax.devices()[:NCORES]
    mesh = Mesh(np.asarray(devices), ("core",))
    n_outs = len(out_names)
    sharded = jax.jit(
        shard_map(_body, mesh=mesh,
                  in_specs=(PartitionSpec("core"),) * (n_params + n_outs),
                  out_specs=(PartitionSpec("core"),) * n_outs,
                  check_rep=False),
        donate_argnums=tuple(range(n_params, n_params + n_outs)),
        keep_unused=True,
    )
    runner = dict(nc=nc, sharded=sharded, in_names=in_names,
                  out_names=out_names, zero_shapes=zero_shapes,
                  out_avals=out_avals)
    _CACHE["runner"] = runner
    return runner


def kernel(q, k, v, Wq, bq, Wk, bk, Wv, bv, Wo, bo):
    global LAST_RESULTS
    r = _get_runner()
    in_maps = _prep_in_maps(q, k, v, Wq, bq, Wk, bk, Wv, bv, Wo, bo)

    concat_in = [np.concatenate([m[name] for m in in_maps], axis=0)
                 for name in r["in_names"]]
    concat_zeros = [np.zeros((NCORES * s[0], *s[1:]), d)
                    for (s, d) in r["zero_shapes"]]
    out_arrs = r["sharded"](*concat_in, *concat_zeros)
    results = [
        {name: np.asarray(out_arrs[i]).reshape(NCORES, *r["out_avals"][i].shape)[c]
         for i, name in enumerate(r["out_names"])}
        for c in range(NCORES)
    ]
    LAST_RESULTS = results

    bo_f = np.asarray(bo).astype(np.float32)
    out = np.empty((B, S, D), np.float32)
    for b in range(B):
        out[b] = (results[2 * b]["out"].astype(np.float32)
                  + results[2 * b + 1]["out"].astype(np.float32)
                  + bo_f)
    return out



# revision 6
# speedup vs baseline: 1.9384x; 1.9384x over previous
"""MultiHeadAttention Trainium2 kernel (8 NeuronCores, SPMD).

Sharding: core c = (batch b=c//2, head-group g=c%2). Each core computes
8 of the 16 heads for one batch: Q/K/V projections restricted to the
512 d_model dims of its head group, full attention for those heads, and
a partial output projection. Host sums the two head-group partials per
batch and adds the output bias.

Layout (all bf16 matmuls, f32 accumulation in PSUM):
  x (q/k/v)  : DMAed ONCE into SBUF as 8 chunks of [128 d_in, 2048 tok]
  wq/wk/wv   : (1024, 512)  = W[rows g].T   (d_in, d_out_local)
  wo         : (512, 1024)  = Wo[:, cols g].T
  qhT/khT    : (128, 2048) x4 pairs  (head dims on partitions)
  vh         : (128 tok, 8*65) x16 token blocks; per head 64 v-dims
               plus a ones column so the ctx matmul also produces the
               softmax denominator in PSUM row 64.
  ctxT       : (128, 2048) x4 pairs

The TRN2 PE clock ramps 0.65 -> 1.2 -> 2.4 GHz and needs ~3us of
gap-free execution to reach full rate; every stall resets it. So the
kernel is organized to keep the PE stream dependency-free:
  - x is resident (projections never wait on DMA after the prefetch)
  - weight-stationary projection loops minimize LDWEIGHTS
  - scores run up to 3 PSUM tiles ahead of ctx, so exp latency is hidden
  - softmax normalization (broadcast + approx-reciprocal + mul) happens
    on DVE from an SBUF staging copy, off the PE critical path
"""

import numpy as np
import ml_dtypes

BF16 = ml_dtypes.bfloat16

B, S, D, H = 4, 2048, 1024, 16
DH = 64          # head dim
DL = 512         # local d_out (8 heads)
P = 128          # partitions
NCORES = 8
SCALE = 1.0 / np.sqrt(DH)

_CACHE = {}
LAST_RESULTS = None  # stashed BassKernelResults for test harness


def _build_nc():
    import concourse.bass as bass
    from concourse import bacc, mybir
    from concourse.tile import TileContext

    f32 = mybir.dt.float32
    bf16 = mybir.dt.bfloat16

    nc = bacc.Bacc("TRN2", target_bir_lowering=False, debug=False, num_devices=NCORES)

    # x pre-transposed on host: (8 d_in chunks, 128, 2048 tokens)
    xq_d = nc.dram_tensor("xq", (8, P, S), bf16, kind="ExternalInput")
    xk_d = nc.dram_tensor("xk", (8, P, S), bf16, kind="ExternalInput")
    xv_d = nc.dram_tensor("xv", (8, P, S), bf16, kind="ExternalInput")
    wq_d = nc.dram_tensor("wq", (D, DL), bf16, kind="ExternalInput")
    wk_d = nc.dram_tensor("wk", (D, DL), bf16, kind="ExternalInput")
    wv_d = nc.dram_tensor("wv", (D, DL), bf16, kind="ExternalInput")
    wo_d = nc.dram_tensor("wo", (DL, D), bf16, kind="ExternalInput")
    bq_d = nc.dram_tensor("bq", (P, 4), f32, kind="ExternalInput")
    bk_d = nc.dram_tensor("bk", (P, 4), f32, kind="ExternalInput")
    bvb_d = nc.dram_tensor("bvb", (P, 520), bf16, kind="ExternalInput")
    out_d = nc.dram_tensor("out", (S, D), bf16, kind="ExternalOutput")

    Exp = mybir.ActivationFunctionType.Exp

    with TileContext(nc) as tc:
        with tc.tile_pool(name="res", bufs=1) as res:
            # ---------------- resident SBUF ----------------
            wq_sb = res.tile([P, 8, DL], bf16)
            wk_sb = res.tile([P, 8, DL], bf16)
            wv_sb = res.tile([P, 8, DL], bf16)
            wo_sb = res.tile([P, 4, D], bf16)
            bq_sb = res.tile([P, 4], f32)
            bk_sb = res.tile([P, 4], f32)
            bvb_sb = res.tile([P, 520], bf16)

            # two x buffers: A holds xq then xv, B holds xk
            xa = [res.tile([P, S], bf16, name=f"xa{c}") for c in range(8)]
            xb = [res.tile([P, S], bf16, name=f"xb{c}") for c in range(8)]

            qhT = [res.tile([P, S], bf16, name=f"qhT{i}") for i in range(4)]
            khT = [res.tile([P, S], bf16, name=f"khT{i}") for i in range(4)]
            vh = [res.tile([P, 8 * 65], bf16, name=f"vh{i}") for i in range(16)]
            ctxT = [res.tile([P, S], bf16, name=f"ctxT{i}") for i in range(4)]

            # ---------------- prefetch DMAs ----------------
            # weights on the gpsimd queue (wq first: needed earliest)
            nc.gpsimd.dma_start(out=wq_sb, in_=wq_d.ap().rearrange("(c p) d -> p c d", p=P))
            nc.gpsimd.dma_start(out=bq_sb, in_=bq_d.ap())
            nc.gpsimd.dma_start(out=wk_sb, in_=wk_d.ap().rearrange("(c p) d -> p c d", p=P))
            nc.gpsimd.dma_start(out=bk_sb, in_=bk_d.ap())
            nc.gpsimd.dma_start(out=wv_sb, in_=wv_d.ap().rearrange("(c p) d -> p c d", p=P))
            nc.gpsimd.dma_start(out=bvb_sb, in_=bvb_d.ap())
            nc.gpsimd.dma_start(out=wo_sb, in_=wo_d.ap().rearrange("(c p) d -> p c d", p=P))
            # x chunks round-robin over three queues; xq first, then xk, xv
            qs = [nc.sync, nc.scalar]
            for c in range(8):
                qs[c % 2].dma_start(out=xa[c], in_=xq_d.ap()[c])
            for c in range(8):
                qs[c % 2].dma_start(out=xb[c], in_=xk_d.ap()[c])

            # ---------------- phase 1: projections ----------------
            with tc.tile_pool(name="pj", bufs=8, space="PSUM") as pj:
                def proj_qk(x_sb, w_sb, b_sb, dst):
                    # weight-stationary: one LDW per (pair, c-chunk), the four
                    # token tiles stream against it
                    for p in range(4):
                        ps = [pj.tile([P, 512], f32, name="pjt", tag="pj")
                              for _ in range(4)]
                        for c in range(8):
                            for t in range(4):
                                nc.tensor.matmul(
                                    ps[t], lhsT=w_sb[:, c, p * P:(p + 1) * P],
                                    rhs=x_sb[c][:, t * 512:(t + 1) * 512],
                                    start=(c == 0), stop=(c == 7))
                        for t in range(4):
                            nc.vector.tensor_scalar_add(
                                out=dst[p][:, t * 512:(t + 1) * 512],
                                in0=ps[t], scalar1=b_sb[:, p:p + 1])

                proj_qk(xa, wq_sb, bq_sb, qhT)
                # xv reuses the xa tiles: emitted AFTER the Q projection so
                # each chunk's DMA waits (WAR) for Q-proj's reads, and V-proj
                # below waits (RAW) for the DMA. Overlaps the K projection.
                for c in range(8):
                    qs[c % 2].dma_start(out=xa[c], in_=xv_d.ap()[c])
                proj_qk(xb, wk_sb, bk_sb, khT)

                # V: x-stationary so the output lands token-partitioned
                for tb in range(16):
                    psv = pj.tile([P, 512], f32, name="psv", tag="pj")
                    for c in range(8):
                        nc.tensor.matmul(
                            psv, lhsT=xa[c][:, tb * P:(tb + 1) * P],
                            rhs=wv_sb[:, c, :],
                            start=(c == 0), stop=(c == 7))
                    vt = vh[tb].rearrange("p (h e) -> p h e", e=65)
                    nc.vector.tensor_copy(
                        vt[:, :, 0:64],
                        psv.rearrange("p (h e) -> p h e", e=64))
                    nc.gpsimd.memset(vt[:, :, 64:65], 1.0)
                    nc.vector.tensor_add(vh[tb], vh[tb], bvb_sb)

            # ---------------- phase 2: attention ----------------
            with tc.tile_pool(name="sc", bufs=3, space="PSUM") as sc, \
                 tc.tile_pool(name="cx", bufs=1, space="PSUM") as cx, \
                 tc.tile_pool(name="pt", bufs=4) as ptp, \
                 tc.tile_pool(name="st", bufs=2) as stp, \
                 tc.tile_pool(name="sm", bufs=1) as sm:
                for p in range(4):
                    for hh in range(2):
                        h = 2 * p + hh
                        po = 64 * hh
                        for qh in range(2):
                            q0 = qh * 1024
                            cps = cx.tile([P, 1024], f32, name="cps", tag="cx")
                            for kb in range(16):
                                sps = sc.tile([P, 1024], f32, name="sps", tag="s")
                                for j in range(2):
                                    nc.tensor.matmul(
                                        sps[:, j * 512:(j + 1) * 512],
                                        lhsT=khT[p][po:po + 64, kb * P:(kb + 1) * P],
                                        rhs=qhT[p][po:po + 64, q0 + j * 512:q0 + (j + 1) * 512],
                                        start=True, stop=True)
                                pt = ptp.tile([P, 1024], bf16, name="ptt", tag="pt")
                                nc.scalar.activation(pt, sps, Exp, scale=SCALE)
                                for j in range(2):
                                    nc.tensor.matmul(
                                        cps[0:65, j * 512:(j + 1) * 512],
                                        lhsT=vh[kb][:, 65 * h:65 * h + 65],
                                        rhs=pt[:, j * 512:(j + 1) * 512],
                                        start=(kb == 0), stop=(kb == 15))
                            # stage ctx+denominator to SBUF, freeing the PSUM
                            # bank; normalize on DVE off the PE critical path
                            stg = stp.tile([P, 1024], f32, name="stg", tag="st")
                            nc.vector.tensor_copy(stg[0:65, :], cps[0:65, :])
                            rc = sm.tile([1, 1024], f32, name="rc", tag="rc")
                            nc.gpsimd.dma_start(out=rc, in_=stg[64:65, :])
                            step = (list(rc.ap[1])[0]
                                    if hasattr(rc.ap[1], "__iter__") else 1)
                            bc = sm.tile([64, 1024], f32, name="bc", tag="bc")
                            nc.gpsimd.dma_start(
                                out=bc,
                                in_=bass.AP(tensor=rc.tensor, offset=rc.offset,
                                            ap=[[1, 1], [0, 64], [step, 1024]]))
                            rb = sm.tile([64, 1024], f32, name="rb", tag="rb")
                            nc.vector.reciprocal_approx_fast(rb, bc)
                            nc.vector.tensor_mul(
                                ctxT[p][po:po + 64, q0:q0 + 1024], stg[0:64, :], rb)

            # ---------------- phase 3: output projection ----------------
            with tc.tile_pool(name="po", bufs=3, space="PSUM") as pop, \
                 tc.tile_pool(name="ot", bufs=3) as otp:
                for qb in range(16):
                    ops = pop.tile([P, D], f32, name="ops", tag="po")
                    for p in range(4):
                        for n in range(2):
                            nc.tensor.matmul(
                                ops[:, n * 512:(n + 1) * 512],
                                lhsT=ctxT[p][:, qb * P:(qb + 1) * P],
                                rhs=wo_sb[:, p, n * 512:(n + 1) * 512],
                                start=(p == 0), stop=(p == 3))
                    ot = otp.tile([P, D], bf16, name="ot", tag="ot")
                    nc.vector.tensor_copy(ot, ops)
                    nc.gpsimd.dma_start(out=out_d.ap()[qb * P:(qb + 1) * P, :], in_=ot)

    nc.finalize()
    return nc


def _prep_in_maps(q, k, v, Wq, bq, Wk, bk, Wv, bv, Wo, bo):
    in_maps = []
    for c in range(NCORES):
        b, g = c // 2, c % 2
        sl = slice(g * DL, (g + 1) * DL)
        bvl = np.asarray(bv)[sl].astype(np.float32)
        bvb = np.zeros(520, np.float32)
        for h in range(8):
            bvb[65 * h:65 * h + 64] = bvl[64 * h:64 * h + 64]
        bvb = np.broadcast_to(bvb, (P, 520))
        def tile_x(x):
            xt = np.ascontiguousarray(np.asarray(x)[b].T).astype(BF16)  # (1024, 2048)
            return xt.reshape(8, P, S)
        in_maps.append({
            "xq": tile_x(q),
            "xk": tile_x(k),
            "xv": tile_x(v),
            "wq": np.ascontiguousarray(np.asarray(Wq)[sl, :].T).astype(BF16),
            "wk": np.ascontiguousarray(np.asarray(Wk)[sl, :].T).astype(BF16),
            "wv": np.ascontiguousarray(np.asarray(Wv)[sl, :].T).astype(BF16),
            "wo": np.ascontiguousarray(np.asarray(Wo)[:, sl].T).astype(BF16),
            "bq": np.ascontiguousarray(np.asarray(bq)[sl].reshape(4, P).T).astype(np.float32),
            "bk": np.ascontiguousarray(np.asarray(bk)[sl].reshape(4, P).T).astype(np.float32),
            "bvb": np.ascontiguousarray(bvb).astype(BF16),
        })
    return in_maps


def _get_runner():
    """Build nc + jitted SPMD executor once; reuse across kernel() calls."""
    if "runner" in _CACHE:
        return _CACHE["runner"]
    import jax
    import jax.numpy as jnp
    from jax.sharding import Mesh, PartitionSpec
    from jax.experimental.shard_map import shard_map
    from concourse import mybir
    from concourse.bass2jax import (_bass_exec_p, install_neuronx_cc_hook,
                                    partition_id_tensor)

    nc = _build_nc()
    install_neuronx_cc_hook()

    partition_name = nc.partition_id_tensor.name if nc.partition_id_tensor else None
    in_names, out_names, out_avals, zero_shapes = [], [], [], []
    for alloc in nc.m.functions[0].allocations:
        if not isinstance(alloc, mybir.MemoryLocationSet):
            continue
        name = alloc.memorylocations[0].name
        if alloc.kind == "ExternalInput":
            if name != partition_name:
                in_names.append(name)
        elif alloc.kind == "ExternalOutput":
            shape = tuple(alloc.tensor_shape)
            dtype = mybir.dt.np(alloc.dtype)
            out_names.append(name)
            out_avals.append(jax.core.ShapedArray(shape, dtype))
            zero_shapes.append((shape, dtype))
    n_params = len(in_names)
    all_in_names = list(in_names) + list(out_names)
    if partition_name is not None:
        all_in_names.append(partition_name)

    def _body(*args):
        operands = list(args)
        if partition_name is not None:
            operands.append(partition_id_tensor())
        outs = _bass_exec_p.bind(
            *operands,
            out_avals=tuple(out_avals),
            in_names=tuple(all_in_names),
            out_names=tuple(out_names),
            lowering_input_output_aliases=(),
            sim_require_finite=True,
            sim_require_nnan=True,
            nc=nc,
        )
        return tuple(outs)

    devices = jax.devices()[:NCORES]
    mesh = Mesh(np.asarray(devices), ("core",))
    n_outs = len(out_names)
    sharded = jax.jit(
        shard_map(_body, mesh=mesh,
                  in_specs=(PartitionSpec("core"),) * (n_params + n_outs),
                  out_specs=(PartitionSpec("core"),) * n_outs,
                  check_rep=False),
        donate_argnums=tuple(range(n_params, n_params + n_outs)),
        keep_unused=True,
    )
    runner = dict(nc=nc, sharded=sharded, in_names=in_names,
                  out_names=out_names, zero_shapes=zero_shapes,
                  out_avals=out_avals)
    _CACHE["runner"] = runner
    return runner


def kernel(q, k, v, Wq, bq, Wk, bk, Wv, bv, Wo, bo):
    global LAST_RESULTS
    r = _get_runner()
    in_maps = _prep_in_maps(q, k, v, Wq, bq, Wk, bk, Wv, bv, Wo, bo)

    concat_in = [np.concatenate([m[name] for m in in_maps], axis=0)
                 for name in r["in_names"]]
    concat_zeros = [np.zeros((NCORES * s[0], *s[1:]), d)
                    for (s, d) in r["zero_shapes"]]
    out_arrs = r["sharded"](*concat_in, *concat_zeros)
    results = [
        {name: np.asarray(out_arrs[i]).reshape(NCORES, *r["out_avals"][i].shape)[c]
         for i, name in enumerate(r["out_names"])}
        for c in range(NCORES)
    ]
    LAST_RESULTS = results

    bo_f = np.asarray(bo).astype(np.float32)
    out = np.empty((B, S, D), np.float32)
    for b in range(B):
        out[b] = (results[2 * b]["out"].astype(np.float32)
                  + results[2 * b + 1]["out"].astype(np.float32)
                  + bo_f)
    return out
